# revision 19
# baseline (speedup 1.0000x reference)
"""MoE FFN (top-2 of 8 experts) Trainium2 kernel.

Strategy: expert-parallel over 8 NeuronCores. The router (logits -> top-2 ->
softmax gates) runs on host in float64 as part of sharding/dispatch; each core
evaluates two expert segments (a slot-packed split of the 8 experts chosen to
minimize the per-core column count), in a feature-major layout (tokens along
the matmul free dimension, expert weights as the stationary operand). Host
combines the expert outputs per token with the gates.

Loop order keeps the PE stationary operand resident across all column chunks
of a segment (kc-outer / chunk-inner), so LDWEIGHTS is amortized over the
whole segment width instead of being paid per matmul.

Self-contained: no imports from the problem directory.
"""

import os
import sys
import types

import numpy as np
import ml_dtypes

import orjson
import concourse.bass as bass
import concourse.tile as tile
from concourse import mybir
from concourse.bass_utils import run_bass_kernel_spmd
import concourse.bass_utils as _bu

# ---------------------------------------------------------------------------
# Toolchain patch: this container's walrus codegen accepts at most ONE
# sync-wait command per instruction, but Tile attaches every required wait to
# the consuming instruction. Rewrite the BIR JSON at the single choke point
# (Bass.to_json_bytes): move all but one wait of a multi-wait instruction onto
# single-wait NoOps inserted immediately before it on the same engine.
# Per-engine streams preserve block order, so a preceding NoOp-with-wait is
# semantically identical to the wait living on the instruction itself.
# ---------------------------------------------------------------------------
if not getattr(bass.Bass, "_mws_patched", False):
    _orig_to_json_bytes = bass.Bass.to_json_bytes
    _mws_ctr = [0]

    def _split_multiwaits(bir):
        for f in bir.get("functions", []):
            for bb in f.get("blocks", []):
                insts = bb.get("instructions", [])
                if not any(
                    len((i.get("sync_info") or {}).get("on_wait") or []) > 1
                    for i in insts
                ):
                    continue
                out = []
                for ins in insts:
                    si = ins.get("sync_info")
                    waits = (si or {}).get("on_wait") or []
                    if len(waits) > 1:
                        for w in waits[:-1]:
                            _mws_ctr[0] += 1
                            out.append({
                                "debug": ins.get("debug", 0),
                                "engine": ins["engine"],
                                "ins": [],
                                "outs": [],
                                "name": f"MWS-{_mws_ctr[0]}",
                                "opcode": "NoOp",
                                "sync_info": {"on_update": [], "on_wait": [w]},
                                "text_hint": "mwsplit",
                            })
                        si["on_wait"] = [waits[-1]]
                    out.append(ins)
                bb["instructions"] = out
        return bir

    def _patched_to_json_bytes(self):
        return orjson.dumps(_split_multiwaits(orjson.loads(_orig_to_json_bytes(self))))

    bass.Bass.to_json_bytes = _patched_to_json_bytes
    bass.Bass._mws_patched = True

# ---------------------------------------------------------------------------
# Optional NTFF profiling shim: the image's `antenv` package lacks
# `axon_hooks`, so trace=True (or BASS_TRACE=1) would crash on import inside
# run_bass_kernel_spmd. Provide the module and register the ctypes hook.
# ---------------------------------------------------------------------------
if "antenv.axon_hooks" not in sys.modules:
    try:
        _mod = types.ModuleType("antenv.axon_hooks")
        _mod._hook = None
        _mod.set_axon_ntff_profile_hook = lambda h: setattr(_mod, "_hook", h)
        _mod.get_axon_ntff_profile_hook = lambda: _mod._hook
        sys.modules["antenv.axon_hooks"] = _mod
        import antenv as _antenv

        _antenv.axon_hooks = _mod
        from trn_agent_boot.trn_boot import _ntff_profile_via_ctypes

        _hook = _ntff_profile_via_ctypes("/opt/axon/libaxon_pjrt.so")
        if _hook is not None:
            _mod.set_axon_ntff_profile_hook(_hook)
        _bu.upload_artifacts = lambda tmpdir: tmpdir  # no cloud bucket here
    except Exception:
        pass

BF16 = ml_dtypes.bfloat16
N_EMBD = 1024
N_EXPERTS = 8
HIDDEN = 4096
N_CORES = 8
KC = N_EMBD // 128   # 8  contraction chunks for layer 1
MH = HIDDEN // 128   # 32 hidden tiles
CT = N_EMBD // 128   # 8  output tiles for layer 2

# Results of the most recent run (test harness reads exec_time_ns from here).
LAST_RUN = {}


def _route_host(xf, gate_w):
    """Top-2 routing in float64. Returns (idx[N,2], gates[N,2]) fp32."""
    logits = xf.astype(np.float64) @ gate_w.astype(np.float64)  # [N, E]
    order = np.argsort(-logits, axis=1, kind="stable")
    top2 = order[:, :2]                                          # [N, 2]
    vals = np.take_along_axis(logits, top2, axis=1)              # [N, 2]
    vals = vals - vals.max(axis=1, keepdims=True)
    ex = np.exp(vals)
    gates = ex / ex.sum(axis=1, keepdims=True)
    return top2.astype(np.int64), gates.astype(np.float32)


def _slot_plan(counts):
    """Pick per-core segment capacities and expert->slot packing.

    Each core runs S segments with capacities caps[0..S-1] (identical across
    cores, so one SPMD program serves all 8 cores); segment s of core i is a
    "slot" that holds a contiguous piece of one expert's token list. An
    expert may occupy several slots (on any cores). The search minimizes
    sum(caps) -- the per-core column count, i.e. the PE work -- over S=2 and
    S=3 slot-size configurations.

    Returns (caps, slot_experts) with slot_experts[s][core] = expert id.
    """
    order = list(np.argsort(-np.asarray(counts), kind="stable"))
    cnts = [int(counts[e]) for e in order]

    def solve(sizes):
        """DFS: assign each expert (desc) a multiset of slots covering its
        count; <=8 slots of each size. Returns per-expert slot multisets."""
        S = len(sizes)
        cands = []
        def gen(i, take):
            if i == S:
                if sum(take) and sum(take[j] * sizes[j] for j in range(S)):
                    cands.append(tuple(take))
                return
            for t in range(5 - sum(take)):
                gen(i + 1, take + [t])
        gen(0, [])
        seen = set()

        def dfs(i, rem):
            if i == len(cnts):
                return []
            key = (i, rem)
            if key in seen:
                return None
            opts = [c for c in cands
                    if all(c[j] <= rem[j] for j in range(S))
                    and sum(c[j] * sizes[j] for j in range(S)) >= cnts[i]]
            opts.sort(key=lambda c: sum(c[j] * sizes[j] for j in range(S)))
            for c in opts[:6]:
                sub = dfs(i + 1, tuple(rem[j] - c[j] for j in range(S)))
                if sub is not None:
                    return [c] + sub
            seen.add(key)
            return None
        return dfs(0, tuple([N_CORES] * S))

    best = None
    # S=2 exact structure (proven optimal for two slots per core)
    for k in range(0, 5):
        m = N_EXPERTS - 2 * k
        top = cnts[:k]
        mid = cnts[k:k + m]
        bot = cnts[k + m:]
        c0 = max([-(-t // 2) for t in top] + [1])
        c1 = max([mx - c0 for mx in mid] + [-(-b // 2) for b in bot] + [1])
        c0 = max(c0, c1)
        if best is None or c0 + c1 < best[0]:
            asg = solve((c0, c1))
            if asg is not None:
                best = (c0 + c1, (c0, c1), asg)
    # S=3 search near the theoretical minimum (time-boxed: the win over the
    # S=2 optimum is ~1% of PE time, so bail out rather than stall the host)
    lo = best[0] if best else 2400
    found3 = None
    budget = [4000]  # max solve() calls
    for tot in range(-(-sum(cnts) // N_CORES), lo, 4):
        for a in range(tot // 3, tot // 3 + 132, 4):
            for b in range((tot - a + 1) // 2, min(a, tot - a) + 1, 4):
                c = tot - a - b
                if c < 64 or c > b:
                    continue
                if budget[0] <= 0:
                    break
                budget[0] -= 1
                asg = solve((a, b, c))
                if asg is not None:
                    found3 = (tot, (a, b, c), asg)
                    break
            if found3 or budget[0] <= 0:
                break
        if found3 or budget[0] <= 0:
            break
    if found3 and found3[0] < best[0]:
        best = found3

    _tot, sizes, asg = best
    # order segments smallest-first: segment 0's X gates the PE start, so
    # the smaller it is, the sooner the stream begins
    perm = sorted(range(len(sizes)), key=lambda j: sizes[j])
    sizes = [sizes[j] for j in perm]
    asg = [tuple(c[j] for j in perm) for c in asg]
    S = len(sizes)
    slot_experts = [[None] * N_CORES for _ in range(S)]
    nxt = [0] * S
    for i, c in enumerate(asg):
        e = int(order[i])
        for sseg in range(S):
            for _ in range(c[sseg]):
                slot_experts[sseg][nxt[sseg]] = e
                nxt[sseg] += 1
    # unused slots (if any) get expert 0 with zero tokens
    for sseg in range(S):
        for i in range(N_CORES):
            if slot_experts[sseg][i] is None:
                slot_experts[sseg][i] = 0
    return list(sizes), slot_experts


def _chunks_for(cap, base_off):
    """Balanced column chunks of <=512 covering [base_off, base_off+cap)."""
    nch = max(1, -(-cap // 512))
    base, rem = divmod(cap, nch)
    out = []
    off = base_off
    for i in range(nch):
        sz = base + (1 if i < rem else 0)
        if sz:
            out.append((off, sz))
        off += sz
    return out


def _build_program(caps, chunk_lists):
    """SPMD Bass program for one core: S expert segments, kc-outer loops."""
    nc = bass.Bass("TRN2", target_bir_lowering=False, debug=False,
                   num_devices=N_CORES)
    f32 = mybir.dt.float32
    bf16 = mybir.dt.bfloat16
    S = len(caps)
    cap = sum(caps)
    offs = [sum(caps[:i]) for i in range(S + 1)]

    xt_d = nc.dram_tensor("xt", [128, KC * cap], bf16, kind="ExternalInput")
    w1_d = nc.dram_tensor("w1t", [S, MH, 128, KC * 128], bf16,
                          kind="ExternalInput")
    w2_d = nc.dram_tensor("w2t", [S, CT, 128, MH * 128], bf16,
                          kind="ExternalInput")
    b1_d = nc.dram_tensor("b1t", [128, S * MH], f32, kind="ExternalInput")
    b2_d = nc.dram_tensor("b2t", [128, S * CT], f32, kind="ExternalInput")
    yt_d = nc.dram_tensor("yt", [CT, 128, cap], bf16, kind="ExternalOutput")

    segs = list(enumerate(chunk_lists))

    with tile.TileContext(nc) as tc:
        with (
            tc.tile_pool(name="big", bufs=1) as big,
            tc.tile_pool(name="w1p", bufs=3) as w1p,
            tc.tile_pool(name="w2p", bufs=2) as w2p,
            tc.tile_pool(name="yp", bufs=4) as yp,
            tc.tile_pool(name="pp", bufs=2, space="PSUM") as pp,
        ):
            xsb = big.tile([128, KC, cap], bf16)
            ht = big.tile([128, MH, cap], bf16)
            warm = big.tile([128, 512], bf16)
            b1sb = big.tile([128, S, MH], f32)
            b2sb = big.tile([128, S, CT], f32)
            nc.sync.dma_start(b1sb[:], b1_d.rearrange("p (s m) -> p s m", s=S))
            nc.sync.dma_start(b2sb[:], b2_d.rearrange("p (s m) -> p s m", s=S))

            # X is laid out kc-major over the whole core window in DRAM
            # ([128, KC, cap]), so each transfer below is a large linear
            # read with multi-KB rows. Segment 0 first, in per-kc pieces so
            # the first kc-outer sweep only waits for its own eighth; the
            # two engine queues (scalar/sync) stream in parallel. Later
            # segments' X is deferred into the mh loop below so it never
            # delays segment 0. gpsimd issues no DMAs at all -- a single
            # gpsimd DMA makes the teardown dge_drain ~4.6us.
            h = KC // 2
            xv = xt_d.rearrange("p (k t) -> p k t", k=KC)
            w1sb0 = w1p.tile([128, KC * 128], bf16, tag="w1s", name="w1sb0")
            nc.sync.dma_start(w1sb0[:], w1_d[0, 0])
            c0 = caps[0]
            ch0 = chunk_lists[0]
            if len(ch0) == 2:
                # first kc-pair split per chunk across both queues, so the
                # very first matmul waits on only ~1/8 of segment 0's X
                (o0, s0), (o1, s1) = ch0
                nc.scalar.dma_start(xsb[:, 0:2, o0:o0 + s0],
                                    xv[:, 0:2, o0:o0 + s0])
                nc.sync.dma_start(xsb[:, 0:2, o1:o1 + s1],
                                  xv[:, 0:2, o1:o1 + s1])
                nc.scalar.dma_start(xsb[:, 2:4, 0:c0], xv[:, 2:4, 0:c0])
                nc.scalar.dma_start(xsb[:, 4:6, 0:c0], xv[:, 4:6, 0:c0])
                nc.sync.dma_start(xsb[:, 6:8, 0:c0], xv[:, 6:8, 0:c0])
            else:
                for kk in range(KC // 2):
                    eng = nc.scalar if kk % 2 == 0 else nc.sync
                    eng.dma_start(xsb[:, 2 * kk:2 * kk + 2, 0:c0],
                                  xv[:, 2 * kk:2 * kk + 2, 0:c0])

            # PE warm-up: dummy matmuls on zeroed SBUF keep the tensor
            # engine busy from right after the preamble so the HAM clock
            # gate reaches 8/8 before the first real matmul (otherwise the
            # first ~4us of real work runs at half clock). No DMA deps.
            nc.vector.memset(warm[:], 0)
            wps = pp.tile([128, 512], f32, tag="warm", name="wps")
            for _ in range(3):
                nc.tensor.matmul(wps[:], warm[:, :128], warm[:], start=True,
                                 stop=True)

            # Deferred X staging points: (seg, lo_half, engine, at_mh)
            xstage = []
            for sseg in range(1, S):
                xstage.append((4 + 8 * (sseg - 1), sseg, True))
                xstage.append((8 + 8 * (sseg - 1), sseg, False))
            xstage = {at: (sseg, lo) for at, sseg, lo in xstage}

            # ---- Layer 1: ht[h, t] = gelu(sum_c W1[c, h] * x[c, t] + b1[h])
            # Segment-outer so the PE can start as soon as segment 0's X and
            # first slab land; kc-outer / chunk-inner so one stationary tile
            # serves every chunk of the segment.
            for seg, chs in segs:
                for mh in range(MH):
                    if seg == 0 and mh == 0:
                        w1sb = w1sb0
                    else:
                        w1sb = w1p.tile([128, KC * 128], bf16, tag="w1s")
                        nc.sync.dma_start(w1sb[:], w1_d[seg, mh])
                    if seg == 0 and mh in xstage:
                        sseg, lo = xstage[mh]
                        o0, o1 = offs[sseg], offs[sseg + 1]
                        if lo:
                            nc.scalar.dma_start(xsb[:, :h, o0:o1],
                                                xv[:, :h, o0:o1])
                        else:
                            nc.sync.dma_start(xsb[:, h:, o0:o1],
                                              xv[:, h:, o0:o1])
                    pss = [pp.tile([128, 512], f32, tag=f"ps{i}", name=f"ps{i}")
                           for i in range(len(chs))]
                    for kc in range(KC):
                        wk = w1sb[:, kc * 128:(kc + 1) * 128]
                        for i, (off, sz) in enumerate(chs):
                            nc.tensor.matmul(
                                pss[i][:, :sz],
                                wk,
                                xsb[:, kc, off:off + sz],
                                start=(kc == 0),
                                stop=(kc == KC - 1),
                            )
                    for i, (off, sz) in enumerate(chs):
                        nc.scalar.activation(
                            ht[:, mh, off:off + sz],
                            pss[i][:, :sz],
                            mybir.ActivationFunctionType.Gelu,
                            bias=b1sb[:, seg, mh:mh + 1],
                        )

            # ---- Layer 2: y[c, t] = sum_h W2[h, c] * ht[h, t] + b2[c]
            # reversed: ends on the smallest segment -> shortest drain tail
            for seg, chs in reversed(segs):
                for ct in range(CT):
                    w2sb = w2p.tile([128, MH * 128], bf16, tag="w2s")
                    nc.sync.dma_start(w2sb[:], w2_d[seg, ct])
                    pss = [pp.tile([128, 512], f32, tag=f"ps{i}", name=f"ps{i}")
                           for i in range(len(chs))]
                    for kh in range(MH):
                        wk = w2sb[:, kh * 128:(kh + 1) * 128]
                        for i, (off, sz) in enumerate(chs):
                            nc.tensor.matmul(
                                pss[i][:, :sz],
                                wk,
                                ht[:, kh, off:off + sz],
                                start=(kh == 0),
                                stop=(kh == MH - 1),
                            )
                    for i, (off, sz) in enumerate(chs):
                        ysb = yp.tile([128, 512], bf16, name="ysb")
                        nc.vector.tensor_scalar_add(ysb[:, :sz], pss[i][:, :sz],
                                                    b2sb[:, seg, ct:ct + 1])
                        yeng = nc.sync if (ct + i) % 2 == 0 else nc.scalar
                        yeng.dma_start(yt_d[ct, :, off:off + sz], ysb[:, :sz])
    return nc


def _prep_weights(w1, b1, w2, b2):
    """Per-expert weight tensors in the kernel's tiled DRAM layouts."""
    w1t = np.ascontiguousarray(
        w1.astype(BF16).reshape(KC, 128, MH, 128).transpose(2, 1, 0, 3)
        .reshape(MH, 128, KC * 128)
    )
    w2t = np.ascontiguousarray(
        w2.astype(BF16).reshape(MH, 128, CT, 128).transpose(2, 1, 0, 3)
        .reshape(CT, 128, MH * 128)
    )
    b1t = b1.astype(np.float32).reshape(MH, 128).T
    b2t = b2.astype(np.float32).reshape(CT, 128).T
    return w1t, w2t, b1t, b2t


def kernel(x, gate_w, w1, b1, w2, b2):
    x = np.asarray(x)
    B, T, C = x.shape
    N = B * T
    xf = np.ascontiguousarray(x.reshape(N, C).astype(np.float32))
    gate_w = np.asarray(gate_w, dtype=np.float32)
    w1 = np.asarray(w1, dtype=np.float32)
    b1 = np.asarray(b1, dtype=np.float32)
    w2 = np.asarray(w2, dtype=np.float32)
    b2 = np.asarray(b2, dtype=np.float32)

    # --- host router + dispatch (the "all-to-all" of the sharding scheme)
    top2, gates = _route_host(xf, gate_w)
    idx_lists = [np.where((top2 == e).any(axis=1))[0] for e in range(N_EXPERTS)]
    counts = [len(ix) for ix in idx_lists]

    caps, slot_experts = _slot_plan(counts)
    S = len(caps)
    cap = sum(caps)
    offs = [sum(caps[:i]) for i in range(S + 1)]
    chunk_lists = [_chunks_for(caps[s], offs[s]) for s in range(S)]

    # Distribute each expert's tokens over its slots (consecutive pieces).
    slot_tokens = [[None] * N_CORES for _ in range(S)]
    for e in range(N_EXPERTS):
        widths, targets = [], []
        for sseg in range(S):
            for i in range(N_CORES):
                if slot_experts[sseg][i] == e:
                    widths.append(caps[sseg])
                    targets.append((sseg, i))
        n = counts[e]
        nslots = len(widths)
        # balanced split proportional to slot count, capped by width
        base = [min(widths[j], n // nslots) for j in range(nslots)]
        rem = n - sum(base)
        j = 0
        while rem > 0:
            take = min(rem, widths[j] - base[j])
            base[j] += take
            rem -= take
            j += 1
        pos = 0
        for j, (sseg, i) in enumerate(targets):
            slot_tokens[sseg][i] = (e, idx_lists[e][pos:pos + base[j]])
            pos += base[j]
        assert pos == n
    for sseg in range(S):
        for i in range(N_CORES):
            if slot_tokens[sseg][i] is None:
                slot_tokens[sseg][i] = (slot_experts[sseg][i],
                                        np.zeros(0, np.int64))

    # --- per-core inputs
    xf_bf = xf.astype(BF16)
    wprep = [_prep_weights(w1[e], b1[e], w2[e], b2[e]) for e in range(N_EXPERTS)]
    in_maps = []
    core_segs = []  # per core: [(expert, token_idx_array), ...] per segment
    for c in range(N_CORES):
        segs = [slot_tokens[sseg][c] for sseg in range(S)]
        core_segs.append(segs)
        xe = np.zeros((cap, C), BF16)
        for seg, (e, ix) in enumerate(segs):
            xe[offs[seg]: offs[seg] + len(ix)] = xf_bf[ix]
        xt = np.ascontiguousarray(
            xe.reshape(cap, KC, 128).transpose(2, 1, 0).reshape(128, KC * cap))
        in_maps.append({
            "xt": xt,
            "w1t": np.stack([wprep[e][0] for e, _ in segs]),
            "w2t": np.stack([wprep[e][1] for e, _ in segs]),
            "b1t": np.ascontiguousarray(
                np.stack([wprep[e][2] for e, _ in segs], axis=1)
                .reshape(128, S * MH)),
            "b2t": np.ascontiguousarray(
                np.stack([wprep[e][3] for e, _ in segs], axis=1)
                .reshape(128, S * CT)),
        })

    # --- build + run
    nc = _build_program(caps, chunk_lists)
    try:
        res = run_bass_kernel_spmd(nc, in_maps, core_ids=list(range(N_CORES)))
    except Exception:
        # transient PJRT/axon execution errors have been observed; retry once
        res = run_bass_kernel_spmd(nc, in_maps, core_ids=list(range(N_CORES)))
    LAST_RUN["exec_time_ns"] = res.exec_time_ns
    LAST_RUN["mean_exec_time_ns"] = res.mean_exec_time_ns
    LAST_RUN["profile_json"] = res.profile_json
    LAST_RUN["results"] = res

    # --- combine (un-dispatch + gate-weighted sum)
    gate_of = np.zeros((N, N_EXPERTS), np.float32)
    gate_of[np.arange(N), top2[:, 0]] = gates[:, 0]
    gate_of[np.arange(N), top2[:, 1]] = gates[:, 1]
    out = np.zeros((N, C), np.float32)
    for c in range(N_CORES):
        yt = np.asarray(res.results[c]["yt"]).astype(np.float32)  # [CT,128,cap]
        yc = yt.transpose(2, 0, 1).reshape(cap, C)                # [cap, C]
        for seg, (e, ix) in enumerate(core_segs[c]):
            ye = yc[offs[seg]: offs[seg] + len(ix)]
            out[ix] += gate_of[ix, e][:, None] * ye
    return out.reshape(B, T, C).astype(np.float32)
